# revision 5
# baseline (speedup 1.0000x reference)
"""Multi-head self-attention (b=4, s=2048, d_model=1024, h=16, causal) on 8 trn2 cores.

Sharding: core c = (batch b = c//2, head-group g = c%2). Each core computes
8 heads of one batch end-to-end (QKV proj, causal flash-attention, partial
output projection). Host pre-transposes x and the weight slices, and sums the
two partial y's per batch (the W_o all-reduce done host-side at unshard time).

All matmuls run as float32r (full PE rate at N=512 moving, ~1e-4 matmul error).
Attention is computed in transposed layout S^T[k, q] = K @ Q^T so that
attn^T[k, q] feeds the PE directly as the moving operand of out^T = V_aug^T-style
matmuls; V carries an appended ones-column so softmax denominators fall out of
the same matmul (row 64 of the [65, 512] psum output).
"""

import numpy as np

import concourse.bass as bass
import concourse.tile as tile
from concourse import bacc, mybir
from concourse.bass import ts
from concourse.bass_utils import run_bass_kernel_spmd

F32 = mybir.dt.float32
F32R = mybir.dt.float32r

B = 4
S = 2048
DM = 1024
H_TOTAL = 16
DK = 64
N_CORES = 8
H = 8            # heads per core
PAIRS = 4        # head pairs per core (row-packed score matmuls)
NKT = DM // 128  # 8 contraction tiles over d_model
NTCH = S // 512  # 4 token chunks
NTT = S // 128   # 16 token tiles
NQC = S // 512   # 4 query chunks
NKB = S // 128   # 16 key blocks
AUG = DK + 1     # 65: head dim + ones column


def _kernel_body(ctx, tc):
    nc = tc.nc
    xT = nc.dram_tensor("xT", [DM, S], F32R, kind="ExternalInput").ap()
    wqT = nc.dram_tensor("wqT", [DM, H * DK], F32R, kind="ExternalInput").ap()
    wkT = nc.dram_tensor("wkT", [DM, H * DK], F32R, kind="ExternalInput").ap()
    wvT = nc.dram_tensor("wvT", [DM, H * DK], F32R, kind="ExternalInput").ap()
    woT = nc.dram_tensor("woT", [H * DK, DM], F32R, kind="ExternalInput").ap()
    masks = nc.dram_tensor("masks", [4, 128, 512], F32, kind="ExternalInput").ap()
    y = nc.dram_tensor("y", [S, DM], F32, kind="ExternalOutput").ap()

    # ---- outer (whole-kernel) pools ----
    outer = ctx.enter_context(tc.tile_pool(name="outer", bufs=1))
    qT = [outer.tile([128, S], F32R, tag=f"qT{p}", name=f"qT{p}") for p in range(PAIRS)]
    kT = [outer.tile([128, S], F32R, tag=f"kT{p}", name=f"kT{p}") for p in range(PAIRS)]
    v_sb = [outer.tile([128, H * AUG], F32R, tag=f"v{t}", name=f"v{t}") for t in range(NTT)]
    mask_sb = [outer.tile([128, 512], F32, tag=f"m{d}", name=f"m{d}") for d in range(4)]
    for d in range(4):
        nc.sync.dma_start(out=mask_sb[d], in_=masks[d])
    ones1 = outer.tile([128, 1], F32, tag="ones1", name="ones1")
    nc.vector.memset(ones1[:], 1.0)

    # ================= phase 1: projections =================
    with tc.tile_pool(name="wpool", bufs=1) as wp, \
         tc.tile_pool(name="xpool", bufs=12) as xp, \
         tc.tile_pool(name="pproj", bufs=3, space="PSUM") as pp:
        wq = [wp.tile([128, H * DK], F32R, tag=f"wq{i}", name=f"wq{i}") for i in range(NKT)]
        wk = [wp.tile([128, H * DK], F32R, tag=f"wk{i}", name=f"wk{i}") for i in range(NKT)]
        wv = [wp.tile([128, H * DK], F32R, tag=f"wv{i}", name=f"wv{i}") for i in range(NKT)]
        for i in range(NKT):
            nc.sync.dma_start(out=wq[i], in_=wqT[ts(i, 128), :])
            nc.sync.dma_start(out=wk[i], in_=wkT[ts(i, 128), :])
            nc.sync.dma_start(out=wv[i], in_=wvT[ts(i, 128), :])

        for tch in range(NTCH):
            xt = []
            for i in range(NKT):
                t = xp.tile([128, 512], F32R, tag="xt", name="xt")
                nc.sync.dma_start(out=t, in_=xT[ts(i, 128), ts(tch, 512)])
                xt.append(t)
            # qT / kT: out [o=128(pair), t=512] accumulated over 8 d_in tiles
            for p in range(PAIRS):
                psq = pp.tile([128, 512], F32, tag="ps", name="ps")
                for i in range(NKT):
                    nc.tensor.matmul(psq[:], wq[i][:, ts(p, 128)], xt[i][:],
                                     start=(i == 0), stop=(i == NKT - 1))
                nc.vector.tensor_copy(qT[p][:, ts(tch, 512)], psq[:])
                psk = pp.tile([128, 512], F32, tag="ps", name="ps")
                for i in range(NKT):
                    nc.tensor.matmul(psk[:], wk[i][:, ts(p, 128)], xt[i][:],
                                     start=(i == 0), stop=(i == NKT - 1))
                nc.vector.tensor_copy(kT[p][:, ts(tch, 512)], psk[:])
            # v natural layout: out [t=128, d=512] per token tile
            for tt in range(4):
                t_tile = 4 * tch + tt
                psv = pp.tile([128, 512], F32, tag="ps", name="ps")
                for i in range(NKT):
                    nc.tensor.matmul(psv[:], xt[i][:, ts(tt, 128)], wv[i][:],
                                     start=(i == 0), stop=(i == NKT - 1))
                vt = v_sb[t_tile]
                for h in range(H):
                    nc.vector.tensor_copy(vt[:, h * AUG:h * AUG + DK],
                                          psv[:, ts(h, DK)])
                ones_col = vt[:].rearrange("p (h a) -> p h a", a=AUG)[:, :, DK]
                nc.vector.tensor_copy(ones_col, ones1[:].to_broadcast((128, H)))

    # ================= phases 2+3 share the ctx^T tiles =================
    ctxp = ctx.enter_context(tc.tile_pool(name="ctxp", bufs=1))
    ctxT = [ctxp.tile([128, S], F32R, tag=f"ctx{p}", name=f"ctx{p}")
            for p in range(PAIRS)]

    # ================= phase 2: causal attention =================
    with tc.tile_pool(name="attn", bufs=4) as ap_, \
         tc.tile_pool(name="rp", bufs=4) as rp, \
         tc.tile_pool(name="Rp", bufs=4) as Rp, \
         tc.tile_pool(name="pscore", bufs=2, space="PSUM") as ps_s, \
         tc.tile_pool(name="pout", bufs=2, space="PSUM") as ps_o, \
         tc.tile_pool(name="rdram", bufs=4, space="DRAM") as rd_p:
        for p in range(PAIRS):
            ha, hb = 2 * p, 2 * p + 1
            for j in range(NQC):
                nk = 4 * j + 4  # causal: key blocks 0 .. 4j+3
                oa = ps_o.tile([AUG, 512], F32, tag="oa", name="oa")
                ob = ps_o.tile([AUG, 512], F32, tag="ob", name="ob")
                prev = None
                for i in range(nk):
                    sa = ps_s.tile([128, 512], F32, tag="sa", name="sa")
                    sb_ = ps_s.tile([128, 512], F32, tag="sb", name="sb")
                    nc.tensor.matmul(sa[:], kT[p][0:64, ts(i, 128)],
                                     qT[p][0:64, ts(j, 512)], start=True, stop=True)
                    nc.tensor.matmul(sb_[:], kT[p][64:128, ts(i, 128)],
                                     qT[p][64:128, ts(j, 512)], start=True, stop=True)
                    aa = ap_.tile([128, 512], F32R, tag="aa", name="aa")
                    ab = ap_.tile([128, 512], F32R, tag="ab", name="ab")
                    nc.scalar.activation(aa[:], sa[:],
                                         mybir.ActivationFunctionType.Exp, scale=0.125)
                    nc.scalar.activation(ab[:], sb_[:],
                                         mybir.ActivationFunctionType.Exp, scale=0.125)
                    if i >= 4 * j:  # diagonal block: zero the future keys
                        d = i - 4 * j
                        nc.vector.tensor_mul(aa[:], aa[:], mask_sb[d][:])
                        nc.vector.tensor_mul(ab[:], ab[:], mask_sb[d][:])
                    # drain previous block's attn@V while exp(i) runs on ACT
                    if prev is not None:
                        pa, pb, pi = prev
                        nc.tensor.matmul(oa[:], v_sb[pi][:, ha * AUG:(ha + 1) * AUG],
                                         pa[:], start=(pi == 0), stop=False)
                        nc.tensor.matmul(ob[:], v_sb[pi][:, hb * AUG:(hb + 1) * AUG],
                                         pb[:], start=(pi == 0), stop=False)
                    prev = (aa, ab, i)
                pa, pb, pi = prev
                nc.tensor.matmul(oa[:], v_sb[pi][:, ha * AUG:(ha + 1) * AUG],
                                 pa[:], start=(pi == 0), stop=True)
                nc.tensor.matmul(ob[:], v_sb[pi][:, hb * AUG:(hb + 1) * AUG],
                                 pb[:], start=(pi == 0), stop=True)
                # normalize: ctx^T[d, q] = out^T[d, q] / denom[q]
                for (o_ps, row0) in ((oa, 0), (ob, 64)):
                    r = rp.tile([1, 512], F32, tag="r", name="r")
                    nc.vector.reciprocal(r[:], o_ps[DK:AUG, :])
                    rd = rd_p.tile([1, 512], F32, tag="rd", name="rd")
                    nc.sync.dma_start(out=rd, in_=r[:])
                    Rt = Rp.tile([64, 512], F32, tag="R", name="R")
                    nc.sync.dma_start(out=Rt, in_=rd[:].to_broadcast((64, 512)))
                    nc.vector.tensor_mul(ctxT[p][row0:row0 + 64, ts(j, 512)],
                                         o_ps[0:DK, :], Rt[:])

    # ================= phase 3: output projection (partial) =================
    with tc.tile_pool(name="wo", bufs=1) as wop, \
         tc.tile_pool(name="yp", bufs=3) as yp, \
         tc.tile_pool(name="py", bufs=4, space="PSUM") as ps_y:
        wo = [wop.tile([128, DM], F32R, tag=f"wo{p}", name=f"wo{p}") for p in range(PAIRS)]
        for p in range(PAIRS):
            nc.sync.dma_start(out=wo[p], in_=woT[ts(p, 128), :])
        for t in range(NTT):
            ysb = yp.tile([128, DM], F32, tag="y", name="ysb")
            for oc in range(2):
                psy = ps_y.tile([128, 512], F32, tag="py", name="py")
                for p in range(PAIRS):
                    nc.tensor.matmul(psy[:], ctxT[p][:, ts(t, 128)],
                                     wo[p][:, ts(oc, 512)],
                                     start=(p == 0), stop=(p == PAIRS - 1))
                nc.vector.tensor_copy(ysb[:, ts(oc, 512)], psy[:])
            nc.sync.dma_start(out=y[ts(t, 128), :], in_=ysb[:])


_NC_CACHE = None


def _build():
    global _NC_CACHE
    if _NC_CACHE is None:
        from contextlib import ExitStack
        nc = bacc.Bacc("TRN2", target_bir_lowering=False, debug=False,
                       num_devices=N_CORES)
        with tile.TileContext(nc) as tc:
            with ExitStack() as ctx:
                _kernel_body(ctx, tc)
        nc.compile()
        _NC_CACHE = nc
    return _NC_CACHE


def _make_masks():
    # mask[d][K, Q] = 1 if Q >= K + 128*d else 0   (allowed = not future)
    K = np.arange(128)[:, None]
    Q = np.arange(512)[None, :]
    return np.stack([(Q >= K + 128 * d) for d in range(4)]).astype(np.float32)


def kernel(x, W_q, W_k, W_v, W_o, _trace=False, _tmpdir=None):
    x = np.asarray(x, dtype=np.float32)
    masks = _make_masks()
    in_maps = []
    for c in range(N_CORES):
        b, g = divmod(c, 2)
        rows = slice(512 * g, 512 * (g + 1))
        in_maps.append({
            "xT": np.ascontiguousarray(x[b].T),
            "wqT": np.ascontiguousarray(np.asarray(W_q)[rows, :].T),
            "wkT": np.ascontiguousarray(np.asarray(W_k)[rows, :].T),
            "wvT": np.ascontiguousarray(np.asarray(W_v)[rows, :].T),
            "woT": np.ascontiguousarray(np.asarray(W_o)[:, rows].T),
            "masks": masks,
        })
    nc = _build()
    res = run_bass_kernel_spmd(nc, in_maps, core_ids=list(range(N_CORES)),
                               trace=_trace, tmpdir=_tmpdir)
    out = np.stack([res.results[2 * b]["y"] + res.results[2 * b + 1]["y"]
                    for b in range(B)]).astype(np.float32)
    if _trace:
        kernel._last_exec_time_ns = res.exec_time_ns
        kernel._last_results = res
    return out
